# revision 16
# baseline (speedup 1.0000x reference)
"""Inverse 2D Haar wavelet (conv_transpose2d, kernel=stride=2, groups=C) on 8 trn2 cores.

Input  x  [B, 4C, H, W]  (B=16, C=64, H=W=128), subbands a,b,c,d per channel.
Output y  [B, C, 2H, 2W] with, per pixel (h, w):
    y[2h+0, 2w+0] = a - b - c + d      = (a-b) - (c-d) = u - s
    y[2h+0, 2w+1] = a - b + c - d      = (a-b) + (c-d) = u + s
    y[2h+1, 2w+0] = a + b - c - d      = (a+b) - (c+d) = v - t
    y[2h+1, 2w+1] = a + b + c + d      = (a+b) + (c+d) = v + t

Sharding: pure data-parallel over batch, 2 images per core.

Precision: the whole transform runs in fp16 (host converts f32->fp16 going
in and back going out).  fp16 rounding contributes ~4e-4 relative error,
far under the 2e-2 gate, and halves HBM traffic to 33.5 MB/core.  A
DMA-only probe (load+store echo) runs at ~32 us/core for that traffic, so
the kernel is ENGINE-bound, not DMA-bound.

Engine-resource facts this design is built on (all HW-measured here):
  * DVE tensor_tensor holds the DVE/GpSimd SHARED SBUF PORT PAIR for the
    full instruction (it needs the second read port), so any GPSIMD op
    fully serializes against DVE TT work — GPSIMD is NOT an extra
    resource and gets nothing.  (A DVE+GPSIMD split measured as the SUM
    of both engines' times, not the max.)
  * DVE contiguous fp16 TT runs in 2x_1P packed mode: ~1.13 us per
    [128, 2048] op (8 ops/stripe = 9.0 us measured free-run).
  * ACT (scalar engine) has its OWN SBUF ports — truly parallel to DVE —
    and its strided activation-copies cost ~0.7 us fixed + ~0.42 ns/elem,
    so ONE merged copy per row-parity (4096 elems) beats two separate
    2048-elem copies.
  * Per-op self-serialization semaphore waits cost ~0.4-1.3 us/op of
    engine idle; same-engine ordering is in-order issue and needs none.

Per-core layout: SBUF partition p = (image, channel) — 2*64 = 128 — and the
free dim holds (subband k, row-block, w) for a horizontal stripe of hb image
rows. The (image, channel) dims merge into a single stride-contiguous DMA
dim, so each iteration is ONE big load (contiguous hb*w runs per subband per
partition) and ONE big store (2hb contiguous output rows per partition).

Work split per stripe (hb=16: all ops [128, 2048] fp16 unless noted):
    DVE : 8 contiguous TT ops (2x_1P): t=c+d, v=a+b, s=c-d, u=a-b,
          E0=u-s, E1=u+s, O0=v-t, O1=v+t            (~9.0 us)
    ACT : merged strided copy [E0|E1] -> even out rows (w,q)-interleaved,
          merged strided copy [O0|O1] -> odd out rows          (~8.3 us)
    SP  : issues loads AND stores (stores gated on act_sem — the sem-wait
          latency lands on the otherwise-idle SP engine, keeping ACT's
          serial path free of waits)
Stripe heights are tapered ([4,12,16*6,12,4]) so pipeline fill and drain
happen on small stripes — the first store issues ~6 us in, and the final
stripe's latency tail is ~6 us instead of ~20 us at uniform hb=16.

(h, p) output interleave falls out of the free-dim row layout; (w, q) is
done by ACT's merged strided copies.
"""

import numpy as np

B, C, H, W = 16, 64, 128, 128
N_CORES = 8
B_PER_CORE = B // N_CORES

_PROGRAM_CACHE = {}

# Haar subband weights this kernel hardcodes (k, p, q) — must match `filters`.
_HAAR = np.array(
    [
        [[1.0, 1.0], [1.0, 1.0]],     # ll
        [[-1.0, -1.0], [1.0, 1.0]],   # lh
        [[-1.0, 1.0], [-1.0, 1.0]],   # hl
        [[1.0, -1.0], [-1.0, 1.0]],   # hh
    ],
    dtype=np.float32,
)

HEIGHTS = (4, 12, 16, 16, 16, 16, 16, 16, 12, 4)   # tapered stripe rows


def build_program(b2=B_PER_CORE, c=C, h=H, w=W, heights=HEIGHTS, bufs=3,
                  reps=1, drain=False):
    """Per-core Bass program (raw bass, hand-rolled sync).

    Pipeline over the stripe list:
      SP     : load stripe + issue previous stripe's store
      DVE    : 8 contiguous butterfly TT ops -> tmp planes
      ACT    : 2 merged interleave-copies

    `reps` re-runs the whole transform back-to-back inside one program
    (same output, reps x the HBM traffic) — a timing probe used by test.py to
    amplify device-side execution above the axon dispatch floor; the graded
    kernel uses reps=1. With `drain=True` the first load of each rep waits for
    every store of the previous rep, so reps do not share pipeline fill/drain:
    the per-rep marginal then measures a COLD single execution, not sustained
    throughput.
    """
    import concourse.bass as bass
    import concourse.mybir as mybir
    from contextlib import ExitStack

    p_n = b2 * c                 # SBUF partitions used (= 128 at full scale)
    assert p_n <= 128 and sum(heights) == h
    n_it = len(heights)
    hb_max = max(heights)
    fdm = hb_max * w             # max free-dim elements per subband per part

    dt = mybir.dt.float16
    nc = bass.Bass("TRN2", target_bir_lowering=False, debug=False)
    x = nc.dram_tensor("x", [b2, 4 * c, h, w], dt, kind="ExternalInput").ap()
    y = nc.dram_tensor("y", [b2, c, 2 * h, 2 * w], dt, kind="ExternalOutput").ap()

    # [ (bb c), k, h, w ] — (bb c) merges to one DMA dim (stride-contiguous).
    xv = x.rearrange("bb (c k) h w -> (bb c) k h w", k=4)
    # [ (bb c), (h2 w2) ] — per-partition flat output plane.
    yv = y.rearrange("bb c h2 w2 -> (bb c) (h2 w2)")

    in_tiles = [
        nc.alloc_sbuf_tensor(f"tin{j}", [p_n, 4 * fdm], dt).ap() for j in range(bufs)
    ]
    # tmp planes per slot (plane stride fdm): 0=t 1=v 2=s 3=u 4=E0 5=E1 6=O0 7=O1.
    # Triple-buffered: ACT runs behind DVE with two iterations of slack, so
    # DVE's WAR wait is pre-satisfied in steady state (no sem-latency stall).
    tmps = [nc.alloc_sbuf_tensor(f"ttmp{j}", [p_n, 8 * fdm], dt).ap()
            for j in range(3)]
    out_tiles = [
        nc.alloc_sbuf_tensor(f"tout{j}", [p_n, 4 * fdm], dt).ap() for j in range(bufs)
    ]

    row0 = np.cumsum([0] + list(heights))[:-1]
    stripes = [(int(row0[i]), int(heights[i])) for i in range(n_it)] * reps
    N = len(stripes)
    # Per-op then_inc counts (incs are async completion signals and don't
    # stall the issuing engine; waiters count exact per-op increments so no
    # in-order-completion assumption is needed).
    DV = 8   # dve_sem incs per iteration (one per op)
    AC = 2   # act_sem incs per iteration (one per merged copy)

    with ExitStack() as ctx:
        # Per-slot DMA sems: a single sem shared by two in-flight DMAs is racy
        # (each DMA is 16 independent +1s; a mixed 16 wouldn't mean DMA 0 done).
        load_sems = [
            ctx.enter_context(nc.semaphore(f"load_sem{j}")) for j in range(bufs)
        ]
        store_sems = [
            ctx.enter_context(nc.semaphore(f"store_sem{j}")) for j in range(bufs)
        ]
        dve_sem = ctx.enter_context(nc.semaphore("dve_sem"))
        act_sem = ctx.enter_context(nc.semaphore("act_sem"))
        block = ctx.enter_context(nc.Block())

        def _store(sync, jt):
            r0, hb = stripes[jt]
            fd = hb * w
            # copies of iteration jt complete -> out_tiles[jt % bufs] ready
            sync.wait_ge(act_sem, AC * (jt + 1))
            sync.dma_start(
                out=yv[:, 4 * r0 * w : 4 * (r0 + hb) * w],
                in_=out_tiles[jt % bufs][:, : 4 * fd],
            ).then_inc(store_sems[jt % bufs], 16)

        @block.sync
        def _(sync):
            for it in range(N):
                r0, hb = stripes[it]
                prev_stored = False
                if drain and it > 0 and it % n_it == 0:
                    # rep barrier: issue the previous rep's final store FIRST
                    # (the barrier below waits on it), then wait for every
                    # store of the previous rep to complete.
                    _store(sync, it - 1)
                    prev_stored = True
                    for j in range(bufs):
                        n_st = sum(1 for k in range(it) if k % bufs == j)
                        sync.wait_ge(store_sems[j], 16 * n_st)
                if it >= bufs:
                    # WAR: in_tiles slot consumed once DVE stage-1 (ops 1-4)
                    # of the slot's previous stripe is done.
                    sync.wait_ge(dve_sem, DV * (it - bufs) + 4)
                sync.dma_start(
                    out=in_tiles[it % bufs][:, : 4 * hb * w].rearrange(
                        "p (k hr w) -> p k hr w", k=4, hr=hb
                    ),
                    in_=xv[:, :, r0 : r0 + hb, :],
                ).then_inc(load_sems[it % bufs], 16)
                if it >= 1 and not prev_stored:
                    # issue the previous stripe's store after this load so
                    # the act_sem wait inside _store never delays a load.
                    _store(sync, it - 1)
            _store(sync, N - 1)

        @block.vector
        def _(eng):
            for it in range(N):
                _, hb = stripes[it]
                fd = hb * w
                slot = it % bufs
                tin = in_tiles[slot]
                a, b_, c_, d_ = (tin[:, k * fd : (k + 1) * fd] for k in range(4))
                tm = tmps[it % 3]
                t_, v_, s_, u_, e0, e1, o0, o1 = (
                    tm[:, k * fdm : k * fdm + fd] for k in range(8)
                )
                eng.wait_ge(load_sems[slot], 16 * (it // bufs + 1))
                if it >= 3:
                    # WAR on tmp slot: ACT copied planes of iteration it-3.
                    eng.wait_ge(act_sem, AC * (it - 2))
                eng.tensor_add(t_, c_, d_).then_inc(dve_sem, 1)  # t = c + d
                eng.tensor_add(v_, a, b_).then_inc(dve_sem, 1)   # v = a + b
                eng.tensor_sub(s_, c_, d_).then_inc(dve_sem, 1)  # s = c - d
                eng.tensor_sub(u_, a, b_).then_inc(dve_sem, 1)   # u = a - b
                eng.tensor_sub(e0, u_, s_).then_inc(dve_sem, 1)  # E0 = u - s
                eng.tensor_add(e1, u_, s_).then_inc(dve_sem, 1)  # E1 = u + s
                eng.tensor_sub(o0, v_, t_).then_inc(dve_sem, 1)  # O0 = v - t
                eng.tensor_add(o1, v_, t_).then_inc(dve_sem, 1)  # O1 = v + t

        @block.scalar
        def _(eng):
            for it in range(N):
                r0, hb = stripes[it]
                fd = hb * w
                slot = it % bufs
                tm = tmps[it % 3]
                # [E0|E1] and [O0|O1] plane pairs read as (pl, hr, w) ...
                epair = tm[:, 4 * fdm : 4 * fdm + 2 * fdm].rearrange(
                    "p (pl f) -> p pl f", pl=2
                )[:, :, :fd].rearrange("p pl (hr w) -> p pl hr w", hr=hb)
                opair = tm[:, 6 * fdm : 6 * fdm + 2 * fdm].rearrange(
                    "p (pl f) -> p pl f", pl=2
                )[:, :, :fd].rearrange("p pl (hr w) -> p pl hr w", hr=hb)
                # ... interleaved into even/odd output rows, ONE instruction
                # per parity (write iteration (q, hr, w)): merging the two
                # per-q copies amortizes ACT's ~0.7 us per-instruction cost.
                o5 = out_tiles[slot][:, : 4 * fd].rearrange(
                    "p (hr pp w q) -> p hr pp w q", hr=hb, pp=2, w=w, q=2
                )
                oq_even = o5[:, :, 0, :, :].rearrange("p hr w q -> p q hr w")
                oq_odd = o5[:, :, 1, :, :].rearrange("p hr w q -> p q hr w")
                if it >= bufs:
                    # WAR: slot's previous stripe fully stored.
                    eng.wait_ge(store_sems[slot], 16 * (it // bufs))
                eng.wait_ge(dve_sem, DV * it + 6)   # E0, E1 ready
                eng.copy(oq_even, epair).then_inc(act_sem, 1)
                eng.wait_ge(dve_sem, DV * it + 8)   # O0, O1 ready
                eng.copy(oq_odd, opair).then_inc(act_sem, 1)

    return nc


def _get_program(reps=1, drain=False):
    key = (B_PER_CORE, C, H, W, reps, drain)
    if key not in _PROGRAM_CACHE:
        _PROGRAM_CACHE[key] = build_program(reps=reps, drain=drain)
    return _PROGRAM_CACHE[key]


def core_in_maps(x):
    """Per-core input maps for run_bass_kernel_spmd (fp16, batch-sharded)."""
    x16 = np.ascontiguousarray(x, dtype=np.float16)
    return [
        {"x": np.ascontiguousarray(x16[i * B_PER_CORE : (i + 1) * B_PER_CORE])}
        for i in range(N_CORES)
    ]


def _reference_fallback(x, filters):
    # Generality net for non-Haar filters (not hit by the graded configuration).
    b, c4, h, w = x.shape
    c = c4 // 4
    f = filters.reshape(c, 4, 2, 2)
    xs = x.reshape(b, c, 4, h, w)
    yout = np.einsum("bckhw,ckpq->bchpwq", xs, f)
    return np.ascontiguousarray(yout.reshape(b, c, 2 * h, 2 * w))


def kernel(x, filters):
    x = np.asarray(x, dtype=np.float32)
    filters = np.asarray(filters, dtype=np.float32)

    f = filters.reshape(-1, 4, 2, 2)
    if not (f.shape[0] == C and np.array_equal(f, np.broadcast_to(_HAAR, f.shape))):
        return _reference_fallback(x, filters)

    from concourse.bass_utils import run_bass_kernel_spmd

    nc = _get_program()
    res = run_bass_kernel_spmd(nc, core_in_maps(x), list(range(N_CORES))).results
    return np.concatenate(
        [res[i]["y"] for i in range(N_CORES)], axis=0
    ).astype(np.float32)


# revision 17
# speedup vs baseline: 1.9316x; 1.9316x over previous
"""Inverse 2D Haar wavelet (conv_transpose2d, kernel=stride=2, groups=C) on 8 trn2 cores.

Input  x  [B, 4C, H, W]  (B=16, C=64, H=W=128), subbands a,b,c,d per channel.
Output y  [B, C, 2H, 2W] with, per pixel (h, w):
    y[2h+0, 2w+0] = a - b - c + d      = (a-b) - (c-d) = u - s
    y[2h+0, 2w+1] = a - b + c - d      = (a-b) + (c-d) = u + s
    y[2h+1, 2w+0] = a + b - c - d      = (a+b) - (c+d) = v - t
    y[2h+1, 2w+1] = a + b + c + d      = (a+b) + (c+d) = v + t

Sharding: pure data-parallel over batch, 2 images per core.

Precision: the whole transform runs in fp16 (host converts f32->fp16 going
in and back going out).  fp16 rounding contributes ~4e-4 relative error,
far under the 2e-2 gate, and halves HBM traffic to 33.5 MB/core.  A
DMA-only probe (load+store echo) runs at ~32 us/core for that traffic, so
the kernel is ENGINE-bound, not DMA-bound: the design spreads the
butterfly across all three elementwise-capable engines, which HW probing
shows run concurrently (DVE tensor_tensor uses its dedicated SBUF ports —
the DVE/GpSimd shared-pair lock only applies to 2-port perf modes, which
this kernel never uses).

HW-measured engine rates here (per [128, 2048] fp16 op, free-run):
    DVE contiguous TT (2x_1P packed) ~1.13 us; DVE strided TT ~2.3 us
    GPSIMD TT ~4.0 us (stride-insensitive)
    ACT strided copy ~0.7 us/instr + ~0.83 ns/elem -> merge the two per-q
    copies of a parity into ONE instruction
Cross-engine semaphore waits cost ~0.5-1 us when they actually block, so
consumers are given >= 2 iterations of buffer slack and per-op then_incs
(incs are async and do not stall the issuing engine; waiters count exact
per-op increments so no in-order-completion assumption is needed).

Per-core layout: SBUF partition p = (image, channel) — 2*64 = 128 — and the
free dim holds (subband k, row-block, w) for a horizontal stripe of hb image
rows. The (image, channel) dims merge into a single stride-contiguous DMA
dim, so each iteration is ONE big load (contiguous hb*w runs per subband per
partition) and ONE big store (2hb contiguous output rows per partition).

Work split per stripe (hb=16: ops are [128, 2048] fp16):
    DVE : 6 contiguous TT (2x_1P): t=c+d, v=a+b, s=c-d, u=a-b,
          E0=u-s, E1=u+s                                    (~7.0 us)
    GPS : odd output rows fused+interleaved directly:
          o[2h+1,2w]=v-t, o[2h+1,2w+1]=v+t (strided writes) (~8.0 us)
    ACT : ONE merged strided copy [E0|E1] -> even rows (w,q)-interleaved,
          then issues the store DMA on its own HWDGE ring    (~5.3 us)
Stripe heights are tapered ([4,12,16*6,12,4]) so pipeline fill and drain
happen on small stripes.

(h, p) output interleave falls out of the free-dim row layout; (w, q) is
fused into GPSIMD's strided writes (odd rows) and ACT's merged copy (even).
"""

import numpy as np

B, C, H, W = 16, 64, 128, 128
N_CORES = 8
B_PER_CORE = B // N_CORES

_PROGRAM_CACHE = {}

# Haar subband weights this kernel hardcodes (k, p, q) — must match `filters`.
_HAAR = np.array(
    [
        [[1.0, 1.0], [1.0, 1.0]],     # ll
        [[-1.0, -1.0], [1.0, 1.0]],   # lh
        [[-1.0, 1.0], [-1.0, 1.0]],   # hl
        [[1.0, -1.0], [-1.0, 1.0]],   # hh
    ],
    dtype=np.float32,
)

HEIGHTS = (4, 12, 16, 16, 16, 16, 16, 16, 12, 4)   # tapered stripe rows


def build_program(b2=B_PER_CORE, c=C, h=H, w=W, heights=HEIGHTS, bufs=4,
                  reps=1, drain=False):
    """Per-core Bass program (raw bass, hand-rolled sync).

    Pipeline over the stripe list:
      SP     : load stripe   (one 128-partition DMA)
      DVE    : 6 contiguous butterfly TT ops -> tmp planes
      GPSIMD : odd out rows  (fused strided v-t, v+t)
      ACT    : merged E0,E1 interleave-copy + store DMA

    `reps` re-runs the whole transform back-to-back inside one program
    (same output, reps x the HBM traffic) — a timing probe used by test.py to
    amplify device-side execution above the axon dispatch floor; the graded
    kernel uses reps=1. With `drain=True` the first load of each rep waits for
    every store of the previous rep, so reps do not share pipeline fill/drain:
    the per-rep marginal then measures a COLD single execution, not sustained
    throughput.
    """
    import concourse.bass as bass
    import concourse.mybir as mybir
    from contextlib import ExitStack

    p_n = b2 * c                 # SBUF partitions used (= 128 at full scale)
    assert p_n <= 128 and sum(heights) == h
    n_it = len(heights)
    hb_max = max(heights)
    fdm = hb_max * w             # max free-dim elements per subband per part

    dt = mybir.dt.float16
    nc = bass.Bass("TRN2", target_bir_lowering=False, debug=False)
    x = nc.dram_tensor("x", [b2, 4 * c, h, w], dt, kind="ExternalInput").ap()
    y = nc.dram_tensor("y", [b2, c, 2 * h, 2 * w], dt, kind="ExternalOutput").ap()

    # [ (bb c), k, h, w ] — (bb c) merges to one DMA dim (stride-contiguous).
    xv = x.rearrange("bb (c k) h w -> (bb c) k h w", k=4)
    # [ (bb c), (h2 w2) ] — per-partition flat output plane.
    yv = y.rearrange("bb c h2 w2 -> (bb c) (h2 w2)")

    in_tiles = [
        nc.alloc_sbuf_tensor(f"tin{j}", [p_n, 4 * fdm], dt).ap() for j in range(bufs)
    ]
    # tmp planes per slot (plane stride fdm): 0=t 1=v 2=s 3=u 4=E0 5=E1.
    # Triple-buffered: consumers (GPS, ACT) run behind DVE with slack, so
    # DVE's WAR waits are pre-satisfied in steady state.
    tmps = [nc.alloc_sbuf_tensor(f"ttmp{j}", [p_n, 6 * fdm], dt).ap()
            for j in range(3)]
    out_tiles = [
        nc.alloc_sbuf_tensor(f"tout{j}", [p_n, 4 * fdm], dt).ap() for j in range(bufs)
    ]

    row0 = np.cumsum([0] + list(heights))[:-1]
    stripes = [(int(row0[i]), int(heights[i])) for i in range(n_it)] * reps
    N = len(stripes)
    DV = 6   # dve_sem incs per iteration (one per op)
    GP = 2   # gps_sem incs per iteration
    AC = 1   # act_sem incs per iteration (one merged copy)

    with ExitStack() as ctx:
        # Per-slot DMA sems: a single sem shared by two in-flight DMAs is racy
        # (each DMA is 16 independent +1s; a mixed 16 wouldn't mean DMA 0 done).
        load_sems = [
            ctx.enter_context(nc.semaphore(f"load_sem{j}")) for j in range(bufs)
        ]
        store_sems = [
            ctx.enter_context(nc.semaphore(f"store_sem{j}")) for j in range(bufs)
        ]
        dve_sem = ctx.enter_context(nc.semaphore("dve_sem"))
        gps_sem = ctx.enter_context(nc.semaphore("gps_sem"))
        act_sem = ctx.enter_context(nc.semaphore("act_sem"))
        block = ctx.enter_context(nc.Block())

        @block.sync
        def _(sync):
            for it in range(N):
                r0, hb = stripes[it]
                if drain and it > 0 and it % n_it == 0:
                    # rep barrier: all stores of the previous rep complete
                    for j in range(bufs):
                        n_st = sum(1 for k in range(it) if k % bufs == j)
                        sync.wait_ge(store_sems[j], 16 * n_st)
                if it >= bufs:
                    # WAR: in_tiles slot consumed once DVE stage-1 (ops 1-4)
                    # of the slot's previous stripe is done.
                    sync.wait_ge(dve_sem, DV * (it - bufs) + 4)
                sync.dma_start(
                    out=in_tiles[it % bufs][:, : 4 * hb * w].rearrange(
                        "p (k hr w) -> p k hr w", k=4, hr=hb
                    ),
                    in_=xv[:, :, r0 : r0 + hb, :],
                ).then_inc(load_sems[it % bufs], 16)

        @block.vector
        def _(eng):
            for it in range(N):
                _, hb = stripes[it]
                fd = hb * w
                slot = it % bufs
                tin = in_tiles[slot]
                a, b_, c_, d_ = (tin[:, k * fd : (k + 1) * fd] for k in range(4))
                tm = tmps[it % 3]
                t_, v_, s_, u_, e0, e1 = (
                    tm[:, k * fdm : k * fdm + fd] for k in range(6)
                )
                eng.wait_ge(load_sems[slot], 16 * (it // bufs + 1))
                if it >= 3:
                    # WAR on tmp slot: GPS consumed t,v and ACT copied E0,E1
                    # of iteration it-3 (two iterations of slack).
                    eng.wait_ge(gps_sem, GP * (it - 2))
                    eng.wait_ge(act_sem, AC * (it - 2))
                eng.tensor_add(t_, c_, d_).then_inc(dve_sem, 1)  # t = c + d
                eng.tensor_add(v_, a, b_).then_inc(dve_sem, 1)   # v = a + b
                eng.tensor_sub(s_, c_, d_).then_inc(dve_sem, 1)  # s = c - d
                eng.tensor_sub(u_, a, b_).then_inc(dve_sem, 1)   # u = a - b
                eng.tensor_sub(e0, u_, s_).then_inc(dve_sem, 1)  # E0 = u - s
                eng.tensor_add(e1, u_, s_).then_inc(dve_sem, 1)  # E1 = u + s

        @block.gpsimd
        def _(eng):
            # GPSIMD writes the odd output rows directly with fused strided
            # TT ops (stride costs GPSIMD nothing; ~4 us per op measured).
            for it in range(N):
                _, hb = stripes[it]
                fd = hb * w
                slot = it % bufs
                tm = tmps[it % 3]
                t3 = tm[:, :fd].rearrange("p (hr w) -> p hr w", hr=hb)
                v3 = tm[:, fdm : fdm + fd].rearrange("p (hr w) -> p hr w", hr=hb)
                o5 = out_tiles[slot][:, : 4 * fd].rearrange(
                    "p (hr pp w q) -> p hr pp w q", hr=hb, pp=2, w=w, q=2
                )
                eng.wait_ge(dve_sem, DV * it + 2)   # t, v of this iter ready
                if it >= bufs:
                    # WAR: slot's previous stripe fully stored.
                    eng.wait_ge(store_sems[slot], 16 * (it // bufs))
                eng.tensor_sub(o5[:, :, 1, :, 0], v3, t3).then_inc(gps_sem, 1)
                eng.tensor_add(o5[:, :, 1, :, 1], v3, t3).then_inc(gps_sem, 1)

        @block.scalar
        def _(eng):
            for it in range(N):
                r0, hb = stripes[it]
                fd = hb * w
                slot = it % bufs
                tm = tmps[it % 3]
                # [E0|E1] plane pair read as (pl, hr, w), interleaved into the
                # even output rows in ONE instruction (write iteration
                # (q, hr, w)): merging the two per-q copies amortizes ACT's
                # ~0.7 us per-instruction cost.
                epair = tm[:, 4 * fdm : 6 * fdm].rearrange(
                    "p (pl f) -> p pl f", pl=2
                )[:, :, :fd].rearrange("p pl (hr w) -> p pl hr w", hr=hb)
                o5 = out_tiles[slot][:, : 4 * fd].rearrange(
                    "p (hr pp w q) -> p hr pp w q", hr=hb, pp=2, w=w, q=2
                )
                oq_even = o5[:, :, 0, :, :].rearrange("p hr w q -> p q hr w")
                if it >= bufs:
                    # WAR: slot's previous stripe fully stored.
                    eng.wait_ge(store_sems[slot], 16 * (it // bufs))
                eng.wait_ge(dve_sem, DV * it + 6)   # E0, E1 ready
                eng.copy(oq_even, epair).then_inc(act_sem, 1)
                # The store DMA reads out_tiles asynchronously; wait for our
                # own copy's completion inc and GPSIMD's odd rows before
                # ringing the doorbell.
                eng.wait_ge(act_sem, AC * (it + 1))
                eng.wait_ge(gps_sem, GP * (it + 1))
                eng.dma_start(
                    out=yv[:, 4 * r0 * w : 4 * (r0 + hb) * w],
                    in_=out_tiles[slot][:, : 4 * fd],
                ).then_inc(store_sems[slot], 16)

    return nc


def _get_program(reps=1, drain=False):
    key = (B_PER_CORE, C, H, W, reps, drain)
    if key not in _PROGRAM_CACHE:
        _PROGRAM_CACHE[key] = build_program(reps=reps, drain=drain)
    return _PROGRAM_CACHE[key]


def core_in_maps(x):
    """Per-core input maps for run_bass_kernel_spmd (fp16, batch-sharded)."""
    x16 = np.ascontiguousarray(x, dtype=np.float16)
    return [
        {"x": np.ascontiguousarray(x16[i * B_PER_CORE : (i + 1) * B_PER_CORE])}
        for i in range(N_CORES)
    ]


def _reference_fallback(x, filters):
    # Generality net for non-Haar filters (not hit by the graded configuration).
    b, c4, h, w = x.shape
    c = c4 // 4
    f = filters.reshape(c, 4, 2, 2)
    xs = x.reshape(b, c, 4, h, w)
    yout = np.einsum("bckhw,ckpq->bchpwq", xs, f)
    return np.ascontiguousarray(yout.reshape(b, c, 2 * h, 2 * w))


def kernel(x, filters):
    x = np.asarray(x, dtype=np.float32)
    filters = np.asarray(filters, dtype=np.float32)

    f = filters.reshape(-1, 4, 2, 2)
    if not (f.shape[0] == C and np.array_equal(f, np.broadcast_to(_HAAR, f.shape))):
        return _reference_fallback(x, filters)

    from concourse.bass_utils import run_bass_kernel_spmd

    nc = _get_program()
    res = run_bass_kernel_spmd(nc, core_in_maps(x), list(range(N_CORES))).results
    return np.concatenate(
        [res[i]["y"] for i in range(N_CORES)], axis=0
    ).astype(np.float32)


# revision 18
# speedup vs baseline: 2.1807x; 1.1289x over previous
"""Inverse 2D Haar wavelet (conv_transpose2d, kernel=stride=2, groups=C) on 8 trn2 cores.

Input  x  [B, 4C, H, W]  (B=16, C=64, H=W=128), subbands a,b,c,d per channel.
Output y  [B, C, 2H, 2W] with, per pixel (h, w):
    y[2h+0, 2w+0] = a - b - c + d      = (a-b) - (c-d) = u - s
    y[2h+0, 2w+1] = a - b + c - d      = (a-b) + (c-d) = u + s
    y[2h+1, 2w+0] = a + b - c - d      = (a+b) - (c+d) = v - t
    y[2h+1, 2w+1] = a + b + c + d      = (a+b) + (c+d) = v + t

Sharding: pure data-parallel over batch, 2 images per core.

Precision: the whole transform runs in fp16 (host converts f32->fp16 going
in and back going out).  fp16 rounding contributes ~4e-4 relative error,
far under the 2e-2 gate, and halves HBM traffic to 33.5 MB/core.  A
DMA-only probe (load+store echo) runs at ~32 us/core for that traffic, so
the kernel is ENGINE-bound, not DMA-bound: the design spreads the
butterfly across all three elementwise-capable engines, which HW probing
shows run concurrently (DVE tensor_tensor uses its dedicated SBUF ports —
the DVE/GpSimd shared-pair lock only applies to 2-port perf modes, which
this kernel never uses).

HW-measured engine rates here (per [128, 2048] fp16 op, free-run):
    DVE contiguous TT (2x_1P packed) ~1.13 us; DVE strided TT ~2.3 us
    GPSIMD TT ~4.0 us (stride-insensitive)
    ACT strided copy ~0.7 us/instr + ~0.83 ns/elem -> merge the two per-q
    copies of a parity into ONE instruction
Cross-engine semaphore waits cost ~0.5-1 us when they actually block, so
consumers are given >= 2 iterations of buffer slack and per-op then_incs
(incs are async and do not stall the issuing engine; waiters count exact
per-op increments so no in-order-completion assumption is needed).

Per-core layout: SBUF partition p = (image, channel) — 2*64 = 128 — and the
free dim holds (subband k, row-block, w) for a horizontal stripe of hb image
rows. The (image, channel) dims merge into a single stride-contiguous DMA
dim, so each iteration is ONE big load (contiguous hb*w runs per subband per
partition) and ONE big store (2hb contiguous output rows per partition).

Work split per stripe (hb=16: ops are [128, 2048] fp16):
    DVE : 6 contiguous TT (2x_1P): t=c+d, v=a+b, s=c-d, u=a-b,
          E0=u-s, E1=u+s                                    (~7.0 us)
    GPS : odd output rows fused+interleaved directly:
          o[2h+1,2w]=v-t, o[2h+1,2w+1]=v+t (strided writes) (~8.0 us)
    ACT : ONE merged strided copy [E0|E1] -> even rows (w,q)-interleaved,
          then issues the store DMA on its own HWDGE ring    (~5.3 us)
Stripe heights are tapered ([4,12,16*6,12,4]) so pipeline fill and drain
happen on small stripes.

(h, p) output interleave falls out of the free-dim row layout; (w, q) is
fused into GPSIMD's strided writes (odd rows) and ACT's merged copy (even).
"""

import numpy as np

B, C, H, W = 16, 64, 128, 128
N_CORES = 8
B_PER_CORE = B // N_CORES

_PROGRAM_CACHE = {}

# Haar subband weights this kernel hardcodes (k, p, q) — must match `filters`.
_HAAR = np.array(
    [
        [[1.0, 1.0], [1.0, 1.0]],     # ll
        [[-1.0, -1.0], [1.0, 1.0]],   # lh
        [[-1.0, 1.0], [-1.0, 1.0]],   # hl
        [[1.0, -1.0], [-1.0, 1.0]],   # hh
    ],
    dtype=np.float32,
)

HEIGHTS = (4, 12, 16, 16, 16, 16, 16, 16, 12, 4)   # tapered stripe rows


def build_program(b2=B_PER_CORE, c=C, h=H, w=W, heights=HEIGHTS, bufs=4,
                  reps=1, drain=False):
    """Per-core Bass program (raw bass, hand-rolled sync).

    Pipeline over the stripe list:
      SP     : load stripe   (one 128-partition DMA)
      DVE    : 6 contiguous butterfly TT ops -> tmp planes
      GPSIMD : odd out rows  (fused strided v-t, v+t)
      ACT    : merged E0,E1 interleave-copy + store DMA

    `reps` re-runs the whole transform back-to-back inside one program
    (same output, reps x the HBM traffic) — a timing probe used by test.py to
    amplify device-side execution above the axon dispatch floor; the graded
    kernel uses reps=1. With `drain=True` the first load of each rep waits for
    every store of the previous rep, so reps do not share pipeline fill/drain:
    the per-rep marginal then measures a COLD single execution, not sustained
    throughput.
    """
    import concourse.bass as bass
    import concourse.mybir as mybir
    from contextlib import ExitStack

    p_n = b2 * c                 # SBUF partitions used (= 128 at full scale)
    assert p_n <= 128 and sum(heights) == h
    n_it = len(heights)
    hb_max = max(heights)
    fdm = hb_max * w             # max free-dim elements per subband per part

    dt = mybir.dt.float16
    nc = bass.Bass("TRN2", target_bir_lowering=False, debug=False)
    x = nc.dram_tensor("x", [b2, 4 * c, h, w], dt, kind="ExternalInput").ap()
    y = nc.dram_tensor("y", [b2, c, 2 * h, 2 * w], dt, kind="ExternalOutput").ap()

    # [ (bb c), k, h, w ] — (bb c) merges to one DMA dim (stride-contiguous).
    xv = x.rearrange("bb (c k) h w -> (bb c) k h w", k=4)
    # [ (bb c), (h2 w2) ] — per-partition flat output plane.
    yv = y.rearrange("bb c h2 w2 -> (bb c) (h2 w2)")

    in_tiles = [
        nc.alloc_sbuf_tensor(f"tin{j}", [p_n, 4 * fdm], dt).ap() for j in range(bufs)
    ]
    # tmp planes per slot (plane stride fdm): 0=t 1=v 2=s 3=u 4=E0 5=E1.
    # Triple-buffered: consumers (GPS, ACT) run behind DVE with slack, so
    # DVE's WAR waits are pre-satisfied in steady state.
    tmps = [nc.alloc_sbuf_tensor(f"ttmp{j}", [p_n, 6 * fdm], dt).ap()
            for j in range(3)]
    out_tiles = [
        nc.alloc_sbuf_tensor(f"tout{j}", [p_n, 4 * fdm], dt).ap() for j in range(bufs)
    ]

    row0 = np.cumsum([0] + list(heights))[:-1]
    stripes = [(int(row0[i]), int(heights[i])) for i in range(n_it)] * reps
    N = len(stripes)
    DV = 8   # dve_sem incs per iteration (one per op)
    GP = 2   # gps_sem incs per iteration
    AC = 1   # act_sem incs per iteration (one merged copy)

    with ExitStack() as ctx:
        # Per-slot DMA sems: a single sem shared by two in-flight DMAs is racy
        # (each DMA is 16 independent +1s; a mixed 16 wouldn't mean DMA 0 done).
        load_sems = [
            ctx.enter_context(nc.semaphore(f"load_sem{j}")) for j in range(bufs)
        ]
        store_sems = [
            ctx.enter_context(nc.semaphore(f"store_sem{j}")) for j in range(bufs)
        ]
        dve_sem = ctx.enter_context(nc.semaphore("dve_sem"))
        gps_sem = ctx.enter_context(nc.semaphore("gps_sem"))
        act_sem = ctx.enter_context(nc.semaphore("act_sem"))
        block = ctx.enter_context(nc.Block())

        @block.sync
        def _(sync):
            for it in range(N):
                r0, hb = stripes[it]
                if drain and it > 0 and it % n_it == 0:
                    # rep barrier: all stores of the previous rep complete
                    for j in range(bufs):
                        n_st = sum(1 for k in range(it) if k % bufs == j)
                        sync.wait_ge(store_sems[j], 16 * n_st)
                if it >= bufs:
                    # WAR: in_tiles slot consumed once DVE stage-1 (ops 1-4)
                    # of the slot's previous stripe is done.
                    sync.wait_ge(dve_sem, DV * (it - bufs) + 4)
                sync.dma_start(
                    out=in_tiles[it % bufs][:, : 4 * hb * w].rearrange(
                        "p (k hr w) -> p k hr w", k=4, hr=hb
                    ),
                    in_=xv[:, :, r0 : r0 + hb, :],
                ).then_inc(load_sems[it % bufs], 16)

        @block.vector
        def _(eng):
            for it in range(N):
                _, hb = stripes[it]
                fd = hb * w
                slot = it % bufs
                tin = in_tiles[slot]
                a, b_, c_, d_ = (tin[:, k * fd : (k + 1) * fd] for k in range(4))
                tm = tmps[it % 3]
                t_, v_, s_, u_, e0, e1 = (
                    tm[:, k * fdm : k * fdm + fd] for k in range(6)
                )
                eng.wait_ge(load_sems[slot], 16 * (it // bufs + 1))
                if it >= 3:
                    # WAR on tmp slot: GPS consumed t,v and ACT copied E0,E1
                    # of iteration it-3 (two iterations of slack).
                    eng.wait_ge(gps_sem, GP * (it - 2))
                    eng.wait_ge(act_sem, AC * (it - 2))
                eng.tensor_add(t_, c_, d_).then_inc(dve_sem, 1)  # t = c + d
                eng.tensor_add(v_, a, b_).then_inc(dve_sem, 1)   # v = a + b
                eng.tensor_sub(s_, c_, d_).then_inc(dve_sem, 1)  # s = c - d
                eng.tensor_sub(u_, a, b_).then_inc(dve_sem, 1)   # u = a - b
                eng.tensor_sub(e0, u_, s_).then_inc(dve_sem, 1)  # E0 = u - s
                eng.tensor_add(e1, u_, s_).then_inc(dve_sem, 1)  # E1 = u + s
                # Row-rebalance: DVE takes the last 2 odd output rows off
                # GPSIMD (GPS ~4.0 us/op vs DVE ~2.3 us strided equivalent;
                # this evens the two engines at ~7.5 us/stripe).
                hg = hb - 2
                t3 = tm[:, :fd].rearrange("p (hr w) -> p hr w", hr=hb)
                v3 = tm[:, fdm : fdm + fd].rearrange("p (hr w) -> p hr w", hr=hb)
                o5 = out_tiles[slot][:, : 4 * fd].rearrange(
                    "p (hr pp w q) -> p hr pp w q", hr=hb, pp=2, w=w, q=2
                )
                if it >= bufs:
                    # WAR: slot's previous stripe fully stored (DVE writes
                    # out_tiles for these tail rows).
                    eng.wait_ge(store_sems[slot], 16 * (it // bufs))
                eng.tensor_sub(
                    o5[:, hg:, 1, :, 0], v3[:, hg:], t3[:, hg:]
                ).then_inc(dve_sem, 1)
                eng.tensor_add(
                    o5[:, hg:, 1, :, 1], v3[:, hg:], t3[:, hg:]
                ).then_inc(dve_sem, 1)

        @block.gpsimd
        def _(eng):
            # GPSIMD writes the odd output rows directly with fused strided
            # TT ops (stride costs GPSIMD nothing; ~4 us per op measured).
            for it in range(N):
                _, hb = stripes[it]
                fd = hb * w
                slot = it % bufs
                tm = tmps[it % 3]
                hg = hb - 2
                t3 = tm[:, :fd].rearrange("p (hr w) -> p hr w", hr=hb)
                v3 = tm[:, fdm : fdm + fd].rearrange("p (hr w) -> p hr w", hr=hb)
                o5 = out_tiles[slot][:, : 4 * fd].rearrange(
                    "p (hr pp w q) -> p hr pp w q", hr=hb, pp=2, w=w, q=2
                )
                eng.wait_ge(dve_sem, DV * it + 2)   # t, v of this iter ready
                if it >= bufs:
                    # WAR: slot's previous stripe fully stored.
                    eng.wait_ge(store_sems[slot], 16 * (it // bufs))
                eng.tensor_sub(
                    o5[:, :hg, 1, :, 0], v3[:, :hg], t3[:, :hg]
                ).then_inc(gps_sem, 1)
                eng.tensor_add(
                    o5[:, :hg, 1, :, 1], v3[:, :hg], t3[:, :hg]
                ).then_inc(gps_sem, 1)

        @block.scalar
        def _(eng):
            for it in range(N):
                r0, hb = stripes[it]
                fd = hb * w
                slot = it % bufs
                tm = tmps[it % 3]
                # [E0|E1] plane pair read as (pl, hr, w), interleaved into the
                # even output rows in ONE instruction (write iteration
                # (q, hr, w)): merging the two per-q copies amortizes ACT's
                # ~0.7 us per-instruction cost.
                epair = tm[:, 4 * fdm : 6 * fdm].rearrange(
                    "p (pl f) -> p pl f", pl=2
                )[:, :, :fd].rearrange("p pl (hr w) -> p pl hr w", hr=hb)
                o5 = out_tiles[slot][:, : 4 * fd].rearrange(
                    "p (hr pp w q) -> p hr pp w q", hr=hb, pp=2, w=w, q=2
                )
                oq_even = o5[:, :, 0, :, :].rearrange("p hr w q -> p q hr w")
                if it >= bufs:
                    # WAR: slot's previous stripe fully stored.
                    eng.wait_ge(store_sems[slot], 16 * (it // bufs))
                eng.wait_ge(dve_sem, DV * it + 6)   # E0, E1 ready
                eng.copy(oq_even, epair).then_inc(act_sem, 1)
                # The store DMA reads out_tiles asynchronously; wait for our
                # own copy's completion inc and GPSIMD's odd rows before
                # ringing the doorbell.
                eng.wait_ge(act_sem, AC * (it + 1))
                eng.wait_ge(gps_sem, GP * (it + 1))
                eng.wait_ge(dve_sem, DV * (it + 1))  # DVE odd-row tail done
                eng.dma_start(
                    out=yv[:, 4 * r0 * w : 4 * (r0 + hb) * w],
                    in_=out_tiles[slot][:, : 4 * fd],
                ).then_inc(store_sems[slot], 16)

    return nc


def _get_program(reps=1, drain=False):
    key = (B_PER_CORE, C, H, W, reps, drain)
    if key not in _PROGRAM_CACHE:
        _PROGRAM_CACHE[key] = build_program(reps=reps, drain=drain)
    return _PROGRAM_CACHE[key]


def core_in_maps(x):
    """Per-core input maps for run_bass_kernel_spmd (fp16, batch-sharded)."""
    x16 = np.ascontiguousarray(x, dtype=np.float16)
    return [
        {"x": np.ascontiguousarray(x16[i * B_PER_CORE : (i + 1) * B_PER_CORE])}
        for i in range(N_CORES)
    ]


def _reference_fallback(x, filters):
    # Generality net for non-Haar filters (not hit by the graded configuration).
    b, c4, h, w = x.shape
    c = c4 // 4
    f = filters.reshape(c, 4, 2, 2)
    xs = x.reshape(b, c, 4, h, w)
    yout = np.einsum("bckhw,ckpq->bchpwq", xs, f)
    return np.ascontiguousarray(yout.reshape(b, c, 2 * h, 2 * w))


def kernel(x, filters):
    x = np.asarray(x, dtype=np.float32)
    filters = np.asarray(filters, dtype=np.float32)

    f = filters.reshape(-1, 4, 2, 2)
    if not (f.shape[0] == C and np.array_equal(f, np.broadcast_to(_HAAR, f.shape))):
        return _reference_fallback(x, filters)

    from concourse.bass_utils import run_bass_kernel_spmd

    nc = _get_program()
    res = run_bass_kernel_spmd(nc, core_in_maps(x), list(range(N_CORES))).results
    return np.concatenate(
        [res[i]["y"] for i in range(N_CORES)], axis=0
    ).astype(np.float32)
